# revision 6
# baseline (speedup 1.0000x reference)
"""Per-class mean (segment reduce) on 8 Trainium2 NeuronCores.

Algorithm
---------
out[c] = sum_{i: labels[i]==c} features[i] / max(count_c, 1),  C=1000, A=512.

Sharding: rows are split evenly across the 8 cores.  On the host we only
touch the (tiny) labels array: rows are bucketed by class *window*
w = c >> 7 (8 windows of 128 classes; 8*128 = 1024 >= 1000).  Each core
gathers its feature rows window-by-window with SWDGE dma_gather (row i of
the gather order lands on SBUF partition i%128), builds a one-hot
[128 rows x 128 slots] tile per 128-row group with a DVE iota-compare
(slot = label & 127, -1 for padding), and runs ONE fp32 matmul per tile:

    psum_bank[w] += onehot.T @ feat_tile        # [128 classes, 512]

The 8 PSUM banks hold the full [1024, 512] per-core class sums, which are
DMA'd out once.  The host adds the 8 per-core partials and divides by the
global counts (np.bincount of labels) - numerically the same order as the
reference (sum first, divide once, all fp32 matmul math is exact since
the one-hot weights are exactly 0/1 and PSUM accumulates in fp32).

The instruction schedule depends only on the per-window tile counts
(max over cores), so one SPMD program serves all 8 cores; per-core data
(gather indices, slots) are inputs.  Everything is compiled at call time
and memoized per schedule.
"""

import functools
import sys
import types

import numpy as np

N_CORES = 8
NUM_CLASSES = 1000
N_WINDOWS = 8          # class windows of 128 -> 8 PSUM banks
WSIZE = 128
A_DIM = 512
CALL_TILES = 16        # 128-row tiles per dma_gather call (2048 rows)


def _install_axon_hooks_shim():
    """The slim agent image lacks antenv.axon_hooks; concourse imports it
    when tracing.  Provide a no-op fallback so imports never fail."""
    if "antenv.axon_hooks" in sys.modules:
        return
    try:
        from trn_agent_boot.trn_boot import _ntff_profile_via_ctypes
        hook = _ntff_profile_via_ctypes("/opt/axon/libaxon_pjrt.so")
    except Exception:
        hook = None
    mod = types.ModuleType("antenv.axon_hooks")
    mod.get_axon_ntff_profile_hook = lambda: hook
    mod.set_axon_ntff_profile_hook = lambda h: None
    sys.modules["antenv.axon_hooks"] = mod
    # tracing tries to upload artifacts to shared storage; keep it local
    try:
        import concourse.bass_utils as _bu
        _bu.upload_artifacts = lambda tmpdir: tmpdir
    except Exception:
        pass


@functools.lru_cache(maxsize=4)
def _build_program(n_loc: int, tiles_per_window: tuple):
    """Trace + compile the SPMD Bass program for one schedule."""
    _install_axon_hooks_shim()
    import concourse.bacc as bacc
    import concourse.tile as tile
    from concourse import mybir

    DT = mybir.dt.float32
    T = sum(tiles_per_window)
    n_rows = T * 128
    idx_cols = n_rows // 16

    nc = bacc.Bacc("TRN2", target_bir_lowering=False, debug=False)
    feat = nc.declare_dram_parameter("feat", [n_loc, A_DIM], DT, isOutput=False)
    gidx = nc.declare_dram_parameter("gidx", [128, idx_cols], mybir.dt.int16,
                                     isOutput=False)
    slots = nc.declare_dram_parameter("slots", [128, T], DT, isOutput=False)
    iota_c = nc.declare_dram_parameter("iota_c", [128, 128], DT, isOutput=False)
    out_sums = nc.declare_dram_parameter("out_sums", [N_WINDOWS * 128, A_DIM],
                                         DT, isOutput=True)

    # global tile index -> (window, first-in-window, last-in-window)
    tinfo = []
    for w, tw in enumerate(tiles_per_window):
        for j in range(tw):
            tinfo.append((w, j == 0, j == tw - 1))

    # gather calls: consecutive groups of <= CALL_TILES tiles
    calls = []
    t0 = 0
    while t0 < T:
        calls.append((t0, min(CALL_TILES, T - t0)))
        t0 += min(CALL_TILES, T - t0)

    with tile.TileContext(nc) as tc:
        with (
            tc.tile_pool(name="cst", bufs=1) as cst,
            tc.tile_pool(name="gb", bufs=3) as gb_pool,
            tc.tile_pool(name="oh", bufs=4) as oh_pool,
            tc.tile_pool(name="ps", bufs=1, space="PSUM") as ps_pool,
            tc.tile_pool(name="stg", bufs=1) as stg_pool,
        ):
            gidx_sb = cst.tile([128, idx_cols], mybir.dt.int16, tag="gidx_sb")
            nc.sync.dma_start(gidx_sb[:], gidx[:])
            slots_sb = cst.tile([128, T], DT, tag="slots_sb")
            nc.sync.dma_start(slots_sb[:], slots[:])
            iota_sb = cst.tile([128, 128], DT, tag="iota_sb")
            nc.sync.dma_start(iota_sb[:], iota_c[:])

            psum = []
            for w in range(N_WINDOWS):
                ps_w = ps_pool.tile([128, A_DIM], DT, tag=f"ps_{w}")
                psum.append(ps_w)
            staging = stg_pool.tile([128, N_WINDOWS, A_DIM], DT, tag="stg")

            for c0, ctiles in calls:
                gt = gb_pool.tile([128, CALL_TILES, A_DIM], DT, tag="gt")
                nidx = ctiles * 128
                nc.gpsimd.dma_gather(
                    gt[:, :ctiles, :], feat[:],
                    gidx_sb[:, c0 * 8:c0 * 8 + nidx // 16],
                    nidx, nidx, A_DIM, single_packet=False,
                )
                for j in range(ctiles):
                    t = c0 + j
                    w, first, last = tinfo[t]
                    oh = oh_pool.tile([128, 128], DT, tag="oh")
                    nc.vector.tensor_scalar(
                        oh[:], iota_sb[:], slots_sb[:, t:t + 1], None,
                        op0=mybir.AluOpType.is_equal,
                    )
                    nc.tensor.matmul(psum[w][:], oh[:], gt[:, j, :],
                                     start=first, stop=last)
                    if last:
                        nc.vector.tensor_copy(staging[:, w, :], psum[w][:])

            out_view = out_sums[:].rearrange("(w p) a -> p w a", w=N_WINDOWS)
            nc.sync.dma_start(out_view, staging[:])

    nc.compile()
    return nc


def _schedule(labels_all: np.ndarray):
    """Host-side planning from labels only.  Returns per-core gather
    indices / slot tables plus the shared per-window tile counts."""
    n = labels_all.shape[0]
    n_loc = n // N_CORES
    per_core = []
    counts_w = np.zeros((N_CORES, N_WINDOWS), dtype=np.int64)
    for c in range(N_CORES):
        lab = labels_all[c * n_loc:(c + 1) * n_loc].astype(np.int64)
        win = lab >> 7
        order = np.argsort(win, kind="stable")   # windows ascending, orig order inside
        per_core.append((lab, win, order))
        counts_w[c] = np.bincount(win, minlength=N_WINDOWS)
    tiles_per_window = np.maximum(
        (counts_w.max(axis=0) + WSIZE - 1) // WSIZE, 1
    ).astype(np.int64)
    T = int(tiles_per_window.sum())

    gidx_list, slots_list = [], []
    for c in range(N_CORES):
        lab, win, order = per_core[c]
        g = np.zeros(T * 128, dtype=np.int64)            # padded with row 0
        s = np.full(T * 128, -1.0, dtype=np.float32)     # padded slot -1
        t0 = 0
        pos = 0
        for w in range(N_WINDOWS):
            nw = counts_w[c, w]
            rows = order[pos:pos + nw]
            pos += nw
            g[t0 * 128: t0 * 128 + nw] = rows
            s[t0 * 128: t0 * 128 + nw] = (lab[rows] & 127).astype(np.float32)
            t0 += int(tiles_per_window[w])
        # wrap indices per gather call (16-row column-major within call)
        cols = []
        p0 = 0
        while p0 < T * 128:
            nidx = min(CALL_TILES * 128, T * 128 - p0)
            blk = g[p0:p0 + nidx]
            cols.append(blk.astype(np.int16).reshape(nidx // 16, 16).T)
            p0 += nidx
        gidx_list.append(np.tile(np.concatenate(cols, axis=1), (8, 1)))
        slots_list.append(np.ascontiguousarray(s.reshape(T, 128).T))
    return n_loc, tuple(int(x) for x in tiles_per_window), gidx_list, slots_list


last_run = None  # BassKernelResults of the most recent kernel() call


def kernel(features: np.ndarray, labels: np.ndarray) -> np.ndarray:
    global last_run
    from concourse.bass_utils import run_bass_kernel_spmd

    features = np.asarray(features)
    labels_np = np.asarray(labels)
    n, a = features.shape
    assert a == A_DIM and n % N_CORES == 0

    n_loc, tpw, gidx_list, slots_list = _schedule(labels_np)
    nc = _build_program(n_loc, tpw)

    iota_v = np.tile(np.arange(128, dtype=np.float32)[None, :], (128, 1))
    in_maps = []
    for c in range(N_CORES):
        in_maps.append({
            "feat": np.ascontiguousarray(
                features[c * n_loc:(c + 1) * n_loc]).astype(np.float32,
                                                            copy=False),
            "gidx": gidx_list[c],
            "slots": slots_list[c],
            "iota_c": iota_v,
        })

    res = run_bass_kernel_spmd(nc, in_maps, list(range(N_CORES)))
    last_run = res
    total = np.zeros((N_WINDOWS * 128, A_DIM), dtype=np.float32)
    for c in range(N_CORES):
        total += res.results[c]["out_sums"]

    counts = np.bincount(labels_np.astype(np.int64), minlength=NUM_CLASSES)
    counts = np.maximum(counts[:NUM_CLASSES], 1).astype(np.float32)
    return total[:NUM_CLASSES] / counts[:, None]
